# revision 10
# baseline (speedup 1.0000x reference)
"""Trainium2 Bass kernel for nn_Decoder (show-attend-tell decoder).

Sharding: data-parallel over the caplen-sorted batch across 8 NeuronCores
(8 rows/core). Per core:
  - Phase A: enc' = |w_full|*(feats @ W_enc.T + b_enc) and
    Pemb = emb @ W_ih[:, :E].T + (b_ih + b_hh) for all (t, b), via GEMMs
    (Pemb round-trips through DRAM and is DMA-prefetched per step).
  - Recurrence (T=49): w*relu(enc+att2) == sigma*relu(enc'+att2') with
    |w_full| folded into enc'/att2' and sign into the reduction, so the
    (B,P,A) broadcast-add+relu is one tensor_scalar((x+s) max 0) per
    (b, chunk) on DVE, reduced over A on the PE with sigma as lhsT.
    Softmax skips max-subtraction (e is O(10); exp's accum_out gives the
    sum). sigmoid(x) = 0.5*(1+tanh(x/2)) keeps ACT on one table set.
    Matmul outputs use 32-aligned PSUM rows: b -> row 32*(b%4), col-group
    b//4 for the M=1 reductions (e, ctx).
  - Phase C: scores = H @ W_score.T + b_score over all 392 (t, b) rows,
    mask applied via the copy-out (inactive rows exactly 0).
"""

import functools

import numpy as np
import ml_dtypes

import concourse.bass as bass
import concourse.mybir as mybir
import concourse.tile as tile
from concourse import bacc
from concourse.bass_utils import run_bass_kernel_spmd
from concourse.masks import make_identity

BF16 = ml_dtypes.bfloat16
bf = mybir.dt.bfloat16
f32 = mybir.dt.float32
AF = mybir.ActivationFunctionType
OP = mybir.AluOpType

B, GH, GW, H, E, A, V, MAXCAP = 64, 14, 14, 512, 512, 512, 10000, 50
P = GH * GW          # 196
T = MAXCAP - 1       # 49
NB = 8               # batch rows per core
NC = 8               # cores
TB = T * NB          # 392 (t, b) rows per core
NT_V = 20
VT = V // NT_V       # 500
MT_ROWS = [128, 128, 128, TB - 384]
P1 = P - 128         # 68


def _emit(nc: "bacc.Bacc"):
    def din(name, shape, dtype):
        return nc.dram_tensor(name, list(shape), dtype, kind="ExternalInput")

    featsT_d = din("featsT", [4, 128, NB * P], bf)
    featsP_d = din("featsP", [2, 128, NB * H], bf)
    WencT_d = din("WencT", [4, 128, 4, 128], bf)
    bencP_d = din("bencP", [1, A], bf)
    WdecPT_d = din("WdecPT", [4, 128, 4, 128], bf)
    bdecP_d = din("bdecP", [1, A], bf)
    sigE_d = din("sigE", [128, 4, 8, 8], bf)
    Wrhs_d = din("Wrhs", [8, 128, 4, 512], bf)
    biasg_d = din("biasg", [1, 4 * H], bf)
    WihEmbT_d = din("WihEmbT", [4, 128, 4, 512], bf)
    embT_d = din("embT", [4, 128, TB], bf)
    WscoreT_d = din("WscoreT", [4, 128, V], bf)
    bscore_d = din("bscore", [1, V], bf)
    maskA_d = din("maskA", [NB, T], f32)
    maskMf_d = din("maskMf", [128, 4], f32)
    maskM_d = din("maskM", [1, 4, 128], bf)
    scores_o = nc.dram_tensor("scores_o", [TB, V], f32, kind="ExternalOutput")
    weights_o = nc.dram_tensor("weights_o", [TB, P], f32, kind="ExternalOutput")
    pemb_dram = nc.dram_tensor("pemb_i", [TB, 4 * H], bf, kind="Internal")

    with tile.TileContext(nc) as tc:
        with tc.tile_pool(name="const", bufs=1) as const, \
             tc.tile_pool(name="res", bufs=1) as res:

            # ---------------- constants ----------------
            ident = const.tile([128, 128], f32)
            make_identity(nc, ident[:])
            identB = const.tile([128, 128], bf)
            make_identity(nc, identB[:])
            ones_bf = const.tile([1, 512], bf)
            nc.vector.memset(ones_bf[:], 1.0)
            sigE_sb = const.tile([128, 4, 8, 8], bf)
            nc.sync.dma_start(out=sigE_sb[:], in_=sigE_d.ap())
            bencP_sb = const.tile([1, A], bf)
            nc.sync.dma_start(out=bencP_sb[:], in_=bencP_d.ap())
            bdecP_sb = const.tile([1, A], bf)
            nc.sync.dma_start(out=bdecP_sb[:], in_=bdecP_d.ap())
            biasg_sb = const.tile([1, 4 * H], bf)
            nc.sync.dma_start(out=biasg_sb[:], in_=biasg_d.ap())
            bscore_sb = const.tile([1, V], bf)
            nc.sync.dma_start(out=bscore_sb[:], in_=bscore_d.ap())
            maskA_sb = const.tile([NB, T], f32)
            nc.sync.dma_start(out=maskA_sb[:], in_=maskA_d.ap())
            maskMf_sb = const.tile([128, 4], f32)
            nc.sync.dma_start(out=maskMf_sb[:], in_=maskMf_d.ap())
            maskM_sb = const.tile([1, 4, 128], bf)
            nc.sync.dma_start(out=maskM_sb[:], in_=maskM_d.ap())

            # ---------------- resident tensors ----------------
            WdecPT_sb = res.tile([128, 4, 4, 128], bf)
            nc.sync.dma_start(out=WdecPT_sb[:], in_=WdecPT_d.ap().rearrange("kc k at a -> k kc at a"))
            Wrhs_sb = res.tile([128, 8, 4, 512], bf)
            nc.sync.dma_start(out=Wrhs_sb[:], in_=Wrhs_d.ap().rearrange("kc k nt n -> k kc nt n"))
            featsP_sb = res.tile([128, 2, NB * H], bf)
            nc.sync.dma_start(out=featsP_sb[:, 0, :], in_=featsP_d.ap()[0])
            nc.sync.dma_start(out=featsP_sb[0:P1, 1, :], in_=featsP_d.ap()[1, 0:P1, :])
            encp_sb = res.tile([128, 4, NB * P], bf)
            HT_sb = res.tile([128, 4, NB * (T + 1)], bf)
            nc.vector.memset(HT_sb[:, :, 0:NB], 0.0)
            alpha = res.tile([NB, 256], f32)
            nc.vector.memset(alpha[:, P:256], 0.0)
            aT_z = res.tile([128, 2, NB, NB], bf)
            nc.vector.memset(aT_z[:], 0.0)

            # ---------------- phase A ----------------
            with tc.tile_pool(name="pha", bufs=1) as pha, \
                 tc.tile_pool(name="phat", bufs=2) as phat, \
                 tc.tile_pool(name="phaps", bufs=2, space="PSUM") as phaps:
                featsT_sb = pha.tile([128, 4, NB * P], bf)
                nc.sync.dma_start(out=featsT_sb[:], in_=featsT_d.ap().rearrange("kc k bp -> k kc bp"))
                WencT_sb = pha.tile([128, 4, 4, 128], bf)
                nc.sync.dma_start(out=WencT_sb[:], in_=WencT_d.ap().rearrange("kc k at a -> k kc at a"))
                WihEmbT_sb = pha.tile([128, 4, 4, 512], bf)
                nc.sync.dma_start(out=WihEmbT_sb[:], in_=WihEmbT_d.ap().rearrange("kc k nt n -> k kc nt n"))
                embT_sb = pha.tile([128, 4, TB], bf)
                nc.sync.dma_start(out=embT_sb[:], in_=embT_d.ap().rearrange("kc k tb -> k kc tb"))

                # enc' GEMM: out [a-tile, (b,p)-block]
                NBLK = 4
                BLK = NB * P // NBLK  # 392
                for at in range(4):
                    for nb_ in range(NBLK):
                        ps = phaps.tile([128, 512], f32, tag="encps")
                        sl = slice(nb_ * BLK, (nb_ + 1) * BLK)
                        for kc in range(4):
                            nc.tensor.matmul(ps[:, 0:BLK], lhsT=WencT_sb[:, kc, at, :],
                                             rhs=featsT_sb[:, kc, sl], start=(kc == 0), stop=False)
                        nc.tensor.matmul(ps[:, 0:BLK], lhsT=bencP_sb[0:1, at * 128:(at + 1) * 128],
                                         rhs=ones_bf[0:1, 0:BLK], start=False, stop=True)
                        if (at * NBLK + nb_) % 2 == 0:
                            nc.vector.tensor_copy(encp_sb[:, at, sl], ps[:, 0:BLK])
                        else:
                            nc.scalar.activation(encp_sb[:, at, sl], ps[:, 0:BLK], AF.Copy)

                # Pemb GEMM: out [(t,b) row, gate] + gate bias, to DRAM
                for mt in range(4):
                    mr = MT_ROWS[mt]
                    pet = phat.tile([128, 4 * H], bf, tag="pemb_t")
                    for nt in range(4):
                        ps = phaps.tile([128, 512], f32, tag="pembps")
                        for kc in range(4):
                            nc.tensor.matmul(ps[0:mr, :], lhsT=embT_sb[:, kc, mt * 128:mt * 128 + mr],
                                             rhs=WihEmbT_sb[:, kc, nt, :], start=(kc == 0), stop=False)
                        nc.tensor.matmul(ps[0:mr, :], lhsT=ones_bf[0:1, 0:mr],
                                         rhs=biasg_sb[0:1, nt * 512:(nt + 1) * 512],
                                         start=False, stop=True)
                        if nt % 2 == 0:
                            nc.vector.tensor_copy(pet[0:mr, nt * 512:(nt + 1) * 512], ps[0:mr, :])
                        else:
                            nc.scalar.activation(pet[0:mr, nt * 512:(nt + 1) * 512], ps[0:mr, :], AF.Copy)
                    nc.sync.dma_start(out=pemb_dram.ap()[mt * 128:mt * 128 + mr, :], in_=pet[0:mr, :])

            # ---------------- recurrence ----------------
            with tc.tile_pool(name="work", bufs=2) as work, \
                 tc.tile_pool(name="work1", bufs=1) as work1, \
                 tc.tile_pool(name="pembp", bufs=3) as pembp, \
                 tc.tile_pool(name="pwork", bufs=1, space="PSUM") as pwork, \
                 tc.tile_pool(name="pctx", bufs=1, space="PSUM") as pctx, \
                 tc.tile_pool(name="pgate", bufs=3, space="PSUM") as pgate:

                c_sb = work.tile([NB, H], f32, tag="c_state")
                nc.vector.memset(c_sb[:], 0.0)
                P4 = slice(0, 128, 32)   # the four 32-aligned rows

                for t in range(T):
                    hT_prev = HT_sb[:, :, NB * t:NB * (t + 1)]  # [128, 4, 8] bf16

                    # att2' = h @ WdecP.T + bdecP  -> [a-tile part, (at,b)]
                    att2_ps = pwork.tile([128, 512], f32, tag="att2_ps")
                    for at in range(4):
                        sl = slice(at * NB, (at + 1) * NB)
                        for kc in range(4):
                            nc.tensor.matmul(att2_ps[:, sl], lhsT=WdecPT_sb[:, kc, at, :],
                                             rhs=hT_prev[:, kc, :], start=(kc == 0), stop=False)
                        nc.tensor.matmul(att2_ps[:, sl], lhsT=bdecP_sb[0:1, at * 128:(at + 1) * 128],
                                         rhs=ones_bf[0:1, 0:NB], start=False, stop=True)
                    att2_sb = work.tile([128, 4 * NB], f32, tag="att2_sb")
                    nc.vector.tensor_copy(att2_sb[:], att2_ps[:, 0:4 * NB])

                    # R = relu(enc' + att2') ; e = sigma^T R
                    R_sb = work1.tile([128, 4, NB, P], bf, tag="r_sb")
                    for kc in range(4):
                        for b in range(NB):
                            nc.vector.tensor_scalar(
                                out=R_sb[:, kc, b, :], in0=encp_sb[:, kc, b * P:(b + 1) * P],
                                scalar1=att2_sb[:, kc * NB + b:kc * NB + b + 1], scalar2=0.0,
                                op0=OP.add, op1=OP.max)
                    e_ps = pwork.tile([128, 512], f32, tag="e_ps")
                    for b in range(NB):
                        for kc in range(4):
                            nc.tensor.matmul(e_ps[0:NB, 0:P], lhsT=sigE_sb[:, kc, b, :],
                                             rhs=R_sb[:, kc, b, :],
                                             start=(b == 0 and kc == 0),
                                             stop=(b == NB - 1 and kc == 3))

                    # softmax over p (rows 0..7)
                    expE = work.tile([NB, P], f32, tag="expE")
                    ssum = work.tile([NB, 1], f32, tag="ssum")
                    rsum = work.tile([NB, 1], f32, tag="rsum")
                    nc.scalar.activation(expE[:], e_ps[0:NB, 0:P], AF.Exp,
                                         accum_out=ssum[:])
                    nc.vector.reciprocal(rsum[:], ssum[:])
                    nc.vector.tensor_scalar(out=alpha[:, 0:P], in0=expE[:],
                                            scalar1=rsum[:],
                                            scalar2=maskA_sb[:, t:t + 1],
                                            op0=OP.mult, op1=OP.mult)
                    nc.sync.dma_start(out=weights_o.ap()[t * NB:(t + 1) * NB, :],
                                      in_=alpha[:, 0:P])

                    # alpha^T -> [128, 16] psum (pc1 rows 68:128 are zeros)
                    aT_ps = pwork.tile([128, 16], f32, tag="tp_ps")
                    nc.tensor.transpose(aT_ps[:, 0:NB], alpha[:, 0:128], ident[0:NB, 0:NB])
                    nc.tensor.transpose(aT_ps[:, NB:2 * NB], alpha[:, 128:256], ident[0:NB, 0:NB])
                    # scatter col b into the zero-padded per-b lhsT (col b of block b)
                    for b in range(NB):
                        nc.vector.tensor_copy(aT_z[:, :, b, b:b + 1],
                                              aT_ps[:, b:2 * NB:NB].rearrange("p (c o) -> p c o", o=1))

                    # ctx[b] = alpha_b @ feats_b -> rows 0..7 via zero-padded lhsT
                    ctx_ps = pctx.tile([128, 512], f32, tag="ctx_ps")
                    for b in range(NB):
                        nc.tensor.matmul(ctx_ps[0:NB, :], lhsT=aT_z[:, 0, b, :],
                                         rhs=featsP_sb[:, 0, b * H:(b + 1) * H],
                                         start=(b == 0), stop=False)
                        nc.tensor.matmul(ctx_ps[0:NB, :], lhsT=aT_z[0:P1, 1, b, :],
                                         rhs=featsP_sb[0:P1, 1, b * H:(b + 1) * H],
                                         start=False, stop=(b == NB - 1))
                    ctx_bf = work.tile([NB, H], bf, tag="ctx_bf")
                    nc.vector.tensor_copy(ctx_bf[:], ctx_ps[0:NB, :])

                    # ctx^T -> [128, 4, 8] bf16
                    cT_ps = pwork.tile([128, 32], bf, tag="tp_ps")
                    for kc in range(4):
                        nc.tensor.transpose(cT_ps[:, kc * NB:(kc + 1) * NB],
                                            ctx_bf[:, kc * 128:(kc + 1) * 128],
                                            identB[0:NB, 0:NB])
                    cT_bf = work.tile([128, 4, NB], bf, tag="cT_bf")
                    nc.vector.tensor_copy(cT_bf[:], cT_ps[:, 0:4 * NB].rearrange("p (kc b) -> p kc b", kc=4))

                    # gates: Pemb[t] (prefetched) + Whh@h + Wihc@ctx
                    pembS = pembp.tile([NB, 4 * H], bf, tag="pembS")
                    nc.sync.dma_start(out=pembS[:], in_=pemb_dram.ap()[t * NB:(t + 1) * NB, :])
                    sigt = [None, None, None]
                    sig = [None, None, None]
                    tg = None
                    for nt in range(4):
                        gps = pgate.tile([128, 512], f32, tag="gates_ps")
                        nc.tensor.matmul(gps[0:NB, :], lhsT=identB[0:NB, 0:NB],
                                         rhs=pembS[:, nt * 512:(nt + 1) * 512],
                                         start=True, stop=False)
                        for kc in range(4):
                            nc.tensor.matmul(gps[0:NB, :], lhsT=hT_prev[:, kc, :],
                                             rhs=Wrhs_sb[:, 4 + kc, nt, :], start=False, stop=False)
                        for kc in range(4):
                            nc.tensor.matmul(gps[0:NB, :], lhsT=cT_bf[:, kc, :],
                                             rhs=Wrhs_sb[:, kc, nt, :], start=False, stop=(kc == 3))
                        if nt == 2:
                            tg = work.tile([NB, H], f32, tag="tg")
                            nc.scalar.activation(tg[:], gps[0:NB, :], AF.Tanh)
                        else:
                            j = nt if nt < 2 else 2
                            sigt[j] = work.tile([NB, H], f32, tag=f"sigt{j}", name=f"sigt{j}")
                            nc.scalar.activation(sigt[j][:], gps[0:NB, :], AF.Tanh, scale=0.5)
                            sig[j] = work.tile([NB, H], f32, tag=f"sig{j}", name=f"sig{j}")
                            nc.vector.tensor_scalar(out=sig[j][:], in0=sigt[j][:],
                                                    scalar1=0.5, scalar2=0.5,
                                                    op0=OP.mult, op1=OP.add)

                    # c_new = sf*c + si*tg ; h = so*tanh(c_new)
                    t2 = work.tile([NB, H], f32, tag="t2")
                    nc.vector.tensor_mul(t2[:], sig[1][:], c_sb[:])
                    t1 = work.tile([NB, H], f32, tag="t1")
                    nc.vector.tensor_mul(t1[:], sig[0][:], tg[:])
                    c_sb = work.tile([NB, H], f32, tag="c_state")
                    nc.vector.tensor_add(c_sb[:], t1[:], t2[:])
                    tc_sb = work.tile([NB, H], f32, tag="tc_sb")
                    nc.scalar.activation(tc_sb[:], c_sb[:], AF.Tanh)
                    h_bf = work.tile([NB, H], bf, tag="h_state")
                    nc.vector.tensor_mul(h_bf[:], sig[2][:], tc_sb[:])

                    # h^T -> history
                    hT_ps = pwork.tile([128, 32], bf, tag="tp_ps")
                    for kc in range(4):
                        nc.tensor.transpose(hT_ps[:, kc * NB:(kc + 1) * NB],
                                            h_bf[:, kc * 128:(kc + 1) * 128], identB[0:NB, 0:NB])
                    nc.vector.tensor_copy(
                        HT_sb[:, :, NB * (t + 1):NB * (t + 2)],
                        hT_ps[:, 0:4 * NB].rearrange("p (kc b) -> p kc b", kc=4))

            # ---------------- phase C: scores ----------------
            with tc.tile_pool(name="phc", bufs=4) as phc, \
                 tc.tile_pool(name="phcw", bufs=1) as phcw, \
                 tc.tile_pool(name="phcps", bufs=4, space="PSUM") as phcps:
                WscoreT_sb = phcw.tile([128, 4, V], bf)
                nc.sync.dma_start(out=WscoreT_sb[:], in_=WscoreT_d.ap().rearrange("kc k v -> k kc v"))
                for mt in range(4):
                    mr = MT_ROWS[mt]
                    lsl = slice(NB + 128 * mt, NB + 128 * mt + mr)
                    for nt in range(NT_V):
                        ps = phcps.tile([128, VT], f32, tag="sc_ps")
                        for kc in range(4):
                            nc.tensor.matmul(ps[0:mr, :], lhsT=HT_sb[:, kc, lsl],
                                             rhs=WscoreT_sb[:, kc, nt * VT:(nt + 1) * VT],
                                             start=(kc == 0), stop=False)
                        nc.tensor.matmul(ps[0:mr, :], lhsT=maskM_sb[0:1, mt, 0:mr],
                                         rhs=bscore_sb[0:1, nt * VT:(nt + 1) * VT],
                                         start=False, stop=True)
                        sc_sb = phc.tile([128, VT], f32, tag="sc_sb")
                        if (mt * NT_V + nt) % 2 == 0:
                            nc.vector.tensor_scalar(out=sc_sb[0:mr, :], in0=ps[0:mr, :],
                                                    scalar1=maskMf_sb[0:mr, mt:mt + 1], scalar2=None,
                                                    op0=OP.mult)
                        else:
                            nc.scalar.activation(sc_sb[0:mr, :], ps[0:mr, :], AF.Identity,
                                                 scale=maskMf_sb[0:mr, mt:mt + 1])
                        nc.sync.dma_start(out=scores_o.ap()[128 * mt:128 * mt + mr, nt * VT:(nt + 1) * VT],
                                          in_=sc_sb[0:mr, :])


@functools.lru_cache(maxsize=1)
def _program():
    nc = bacc.Bacc("TRN2", target_bir_lowering=False, debug=False,
                   enable_asserts=True, num_devices=NC)
    _emit(nc)
    nc.compile()
    return nc


def _bf(x):
    return np.ascontiguousarray(np.asarray(x, dtype=np.float32).astype(BF16))


def _prep_core(feats_s, emb_s, dl_c):
    """Per-core host prep. feats_s [8, P, H] f32, emb_s [8, T, E] f32, dl_c [8] int."""
    m = {}
    m["featsT"] = _bf(feats_s.transpose(2, 0, 1).reshape(H, NB * P).reshape(4, 128, NB * P))
    fP = feats_s.transpose(1, 0, 2).reshape(P, NB * H)          # [p, (b,h)]
    fPp = np.zeros((2, 128, NB * H), np.float32)
    fPp[0] = fP[0:128]
    fPp[1, 0:P1] = fP[128:P]
    m["featsP"] = _bf(fPp)
    m["embT"] = _bf(emb_s.transpose(2, 1, 0).reshape(E, TB).reshape(4, 128, TB))
    mask_tb = (np.arange(T)[:, None] < dl_c[None, :]).astype(np.float32)  # [t, b]
    m["maskA"] = np.ascontiguousarray(mask_tb.T)
    flat = np.zeros(512, np.float32)
    flat[:TB] = mask_tb.reshape(TB)
    m["maskMf"] = np.ascontiguousarray(flat.reshape(4, 128).T)  # [128, 4]
    m["maskM"] = _bf(flat.reshape(1, 4, 128))
    return m


def _host_prep(image_features, caps, caplens, emb_table, W_ih, W_hh, b_ih, b_hh,
               W_enc, b_enc, W_dec, b_dec, W_full, b_full, W_score, b_score):
    image_features = np.asarray(image_features)
    caps = np.asarray(caps)
    caplens = np.asarray(caplens)
    cl = caplens[:, 0].astype(np.int64)
    sort_ind = np.argsort(-cl, kind="stable").astype(np.int32)
    cl_s = cl[sort_ind]
    caps_s = np.ascontiguousarray(caps[sort_ind]).astype(np.int32)
    decode_lengths = (cl_s - 1).astype(np.int32)
    feats = np.ascontiguousarray(
        image_features.reshape(B, P, H)[sort_ind]).astype(np.float32)
    emb_all = np.asarray(emb_table)[caps_s[:, :T]]              # [B, T, E]

    w = np.asarray(W_full)[0].astype(np.float32)
    aw = np.abs(w)
    sigma = np.where(w >= 0.0, 1.0, -1.0).astype(np.float32)

    shared = {
        "WencT": _bf((aw[:, None] * np.asarray(W_enc)).T.reshape(4, 128, 4, 128)),
        "bencP": _bf((aw * np.asarray(b_enc))[None, :]),
        "WdecPT": _bf((aw[:, None] * np.asarray(W_dec)).T.reshape(4, 128, 4, 128)),
        "bdecP": _bf((aw * np.asarray(b_dec))[None, :]),
        "sigE": _bf(np.einsum("ck,bm->kcbm", sigma.reshape(4, 128),
                              np.eye(NB, dtype=np.float32)).reshape(128, 4, NB, NB)),
        "Wrhs": _bf(np.concatenate([np.asarray(W_ih)[:, E:].T, np.asarray(W_hh).T], axis=0)
                    .reshape(8, 128, 4, 512)),
        "biasg": _bf((np.asarray(b_ih) + np.asarray(b_hh))[None, :]),
        "WihEmbT": _bf(np.asarray(W_ih)[:, :E].T.reshape(4, 128, 4, 512)),
        "WscoreT": _bf(np.asarray(W_score).T.reshape(4, 128, V)),
        "bscore": _bf(np.asarray(b_score)[None, :]),
    }

    in_maps = []
    for c in range(NC):
        m = _prep_core(feats[c * NB:(c + 1) * NB], emb_all[c * NB:(c + 1) * NB],
                       decode_lengths[c * NB:(c + 1) * NB])
        m.update(shared)
        in_maps.append(m)
    return in_maps, caps_s, decode_lengths, sort_ind


def _assemble(results):
    scores = np.empty((B, T, V), np.float32)
    weights = np.empty((B, T, P), np.float32)
    for c in range(NC):
        sc = results[c]["scores_o"].reshape(T, NB, V).transpose(1, 0, 2)
        wt = results[c]["weights_o"].reshape(T, NB, P).transpose(1, 0, 2)
        scores[c * NB:(c + 1) * NB] = sc
        weights[c * NB:(c + 1) * NB] = wt
    return scores, weights


def kernel(**inputs):
    in_maps, caps_s, decode_lengths, sort_ind = _host_prep(**inputs)
    nc = _program()
    res = run_bass_kernel_spmd(nc, in_maps, core_ids=list(range(NC)))
    scores, weights = _assemble(res.results)
    return scores, caps_s, decode_lengths, weights, sort_ind
